# revision 2
# baseline (speedup 1.0000x reference)
"""GaussianMask kernel for Trainium2 (Bass/Tile), SPMD over 8 NeuronCores.

Problem: X [4,3,512,512] f32 -> K [4,3,24,512,512] f32 where
  K[b,c,k,h,w] = exp(-0.5 * (Xpad[b,c,h+dy,w+dx] - X[b,c,h,w])^2)
for the 24 5x5 neighbor offsets (center excluded), zero padding of 2.

Key algebra exploited on-device:

1. Offset symmetry. Offsets pair up as (dy,dx) <-> (4-dy,4-dx); plane
   23-j is plane j translated by (dy-2, dx-2), and every out-of-bounds
   border pixel of ANY plane equals G := exp(-0.5*X^2). So the device
   computes only planes 0..11 (whose dy is 0..2) plus one G plane; the
   host replicates values into planes 12..23 during unshard (pure data
   movement, no host arithmetic).

2. Gaussian via a single activation. erf'(x) = (2/sqrt(pi))*exp(-x^2),
   so exp(-0.5 d^2) = sqrt(pi)/2 * Derivative_Erf(d/sqrt(2)). The ACT
   free input scale handles 1/sqrt(2); a 4x-mode DVE tensor_scalar_mul
   applies sqrt(pi)/2. This removes the DVE squaring pass entirely:
   per plane the DVE does ONE tensor_sub (2x packed fp16 mode).

Layout (per core): 12 images x 512 rows -> 24 half-images of 256 rows;
3 per core. Partition p holds padded rows 2p..2p+3 (its 2 output rows
plus the dy=0..2 halo) of the 516-wide padded image, fp16. A second
slab loaded at +1 element keeps odd-dx reads 4B-aligned for the DVE
packed mode. Everything is fp16 (ample for the 2e-2 gate; measured l2
rel err ~2e-4), halving both DVE time and store traffic vs f32.

DMA budget: 2 loads + 6 stores = 8 HWDGE DMAs -> no DMA lane reuse.
Each instruction needs at most one cross-engine sem wait (walrus can
encode only one on DVE/DMA instructions).
"""

import numpy as np

import concourse.bass as bass
import concourse.mybir as mybir
import concourse.tile as tile
from concourse.bass_utils import run_bass_kernel_spmd

N_CORES = 8
B, C, H, W = 4, 3, 512, 512
PAD = 2
PW = W + 2 * PAD          # 516 padded width
HALF = 256                # rows per half-image tile
TILES = 3                 # half-images per core
SLAB_ROWS = 4             # padded rows 2p..2p+3 per partition
SLAB = SLAB_ROWS * PW     # 2064 elems per partition per (tile, shift)
IN_ROWS = HALF + 2        # 258 padded rows per half-image
IN_TILE = IN_ROWS * PW    # 133128 elems per half-image input
IN_LEN = TILES * IN_TILE + 8   # +8 pad so the +1-shifted load stays in bounds
NP_DIRECT = 12            # planes computed on device
CHUNK = (NP_DIRECT + 1) * 2 * W   # 13312 out elems per partition per tile
HALF_A = 6 * 2 * W        # 6144: planes 0..5
HALF_B = CHUNK - HALF_A   # 7168: planes 6..11 + G

INV_SQRT2 = 0.7071067811865476
SQRT_PI_OVER_2 = 0.8862269254527580

# planes 0..11 = reference planes 0..11 (idx k for k < 12)
OFFSETS = [(k // 5, k % 5) for k in range(NP_DIRECT)]

_CACHED = None


def _patch_tail_drain():
    """Split the kernel-tail drain's sem waits across one drain per sem.

    Tile attaches every outstanding semaphore wait to a single Drain
    instruction, but walrus' CTRL codegen can only encode a bounded
    number of sync waits per instruction and dies with "Too many sync
    wait commands". One drain per nonzero proc keeps every instruction
    at a single wait.
    """
    from concourse.tile import TileContext
    from concourse.vector_clock import ScopedClock, VectorClock

    if getattr(TileContext, "_tail_drain_patched", False):
        return

    def _drain_and_barrier(self, tick_clock, wait_clock):
        gc = tick_clock.global_clock
        vals = eval(repr(gc).replace("VectorClock", ""))
        for i, v in enumerate(vals):
            if v <= 0:
                continue
            sub = [0] * len(vals)
            sub[i] = v
            drain_inst = self.nc.sync.drain()
            wait_clock.add_sem_waits(
                drain_inst.ins, ScopedClock({None: VectorClock(sub)}))
        self.nc.all_engine_barrier()
        assert self.sems is not None
        popped = self.nc._tile_sem_poison_stack.pop()
        assert popped is self._sem_poison
        self.nc.clear_and_free_semaphores(list(self.sems.allocated().values()))
        self.nc.all_engine_barrier()

    TileContext._drain_and_barrier = _drain_and_barrier
    TileContext._tail_drain_patched = True


def _build_bass():
    _patch_tail_drain()
    nc = bass.Bass("TRN2", target_bir_lowering=False, debug=False,
                   num_devices=N_CORES, dynamic_dma_scratch_size=4096)
    x_h = nc.dram_tensor("x", [IN_LEN], mybir.dt.float16,
                         kind="ExternalInput")
    y_h = nc.dram_tensor("y", [TILES * 128 * CHUNK], mybir.dt.float16,
                         kind="ExternalOutput")

    f16 = mybir.dt.float16
    DErf = mybir.ActivationFunctionType.Derivative_Erf

    with tile.TileContext(nc) as tc:
        with (
            tc.tile_pool(name="slab", bufs=1) as ps,
            tc.tile_pool(name="d", bufs=3) as pd,
            tc.tile_pool(name="out", bufs=3) as po,
        ):
            # slab layout per partition: [tile 3][shift 2][elem 2064]
            slab = ps.tile([128, TILES * 2 * SLAB], f16)
            # tile 0 first so compute can start early; tiles 1-2 second.
            nc.sync.dma_start(
                out=slab[:, 0:2 * SLAB].rearrange("p (s e) -> p s e", e=SLAB),
                in_=bass.AP(x_h, 0, [[2 * PW, 128], [1, 2], [1, SLAB]]))
            nc.sync.dma_start(
                out=slab[:, 2 * SLAB:].rearrange(
                    "p (t s e) -> p t s e", s=2, e=SLAB),
                in_=bass.AP(x_h, IN_TILE,
                            [[2 * PW, 128], [IN_TILE, 2], [1, 2], [1, SLAB]]))

            for t in range(TILES):
                ve = slab[:, (2 * t) * SLAB:(2 * t + 1) * SLAB].rearrange(
                    "p (r c) -> p r c", c=PW)
                vo = slab[:, (2 * t + 1) * SLAB:(2 * t + 2) * SLAB].rearrange(
                    "p (r c) -> p r c", c=PW)
                xi = ve[:, 2:4, 2:2 + W]

                for half in range(2):
                    ncols = HALF_A if half == 0 else HALF_B
                    d = pd.tile([128, HALF_A], f16, tag="d")
                    for j in range(6):
                        dy, dx = OFFSETS[half * 6 + j]
                        if dx % 2 == 0:
                            xj = ve[:, dy:dy + 2, dx:dx + W]
                        else:
                            xj = vo[:, dy:dy + 2, dx - 1:dx - 1 + W]
                        nc.vector.tensor_sub(
                            d[:, j * 1024:(j + 1) * 1024].rearrange(
                                "p (r c) -> p r c", c=W), xj, xi)
                    o = po.tile([128, HALF_B], f16, tag="o")
                    nc.scalar.activation(o[:, 0:HALF_A], d[:],
                                         DErf, scale=INV_SQRT2)
                    if half == 1:
                        nc.scalar.activation(
                            o[:, HALF_A:HALF_B].rearrange(
                                "p (r c) -> p r c", c=W),
                            xi, DErf, scale=INV_SQRT2)
                    nc.vector.tensor_scalar_mul(o[:, 0:ncols], o[:, 0:ncols],
                                                SQRT_PI_OVER_2)
                    dst = bass.AP(y_h, (t * 128) * CHUNK + half * HALF_A,
                                  [[CHUNK, 128], [1, ncols]])
                    nc.sync.dma_start(out=dst, in_=o[:, 0:ncols])
    return nc


def _get_bass():
    global _CACHED
    if _CACHED is None:
        _CACHED = _build_bass()
    return _CACHED


def _shard_inputs(X: np.ndarray):
    """Full X [4,3,512,512] -> per-core flat padded half-image stacks (fp16)."""
    Xi = np.ascontiguousarray(X, dtype=np.float32).reshape(B * C, H, W)
    Xp = np.pad(Xi, ((0, 0), (PAD, PAD), (PAD, PAD))).astype(np.float16)
    in_maps = []
    for c in range(N_CORES):
        arr = np.zeros([IN_LEN], dtype=np.float16)
        for t in range(TILES):
            g = TILES * c + t
            m, r0 = g // 2, (g % 2) * HALF
            arr[t * IN_TILE:(t + 1) * IN_TILE] = \
                Xp[m, r0:r0 + IN_ROWS, :].reshape(-1)
        in_maps.append({"x": arr})
    return in_maps


def _unshard_outputs(results):
    K = np.empty((B * C, 24, H, W), dtype=np.float32)
    G = np.empty((B * C, H, W), dtype=np.float32)
    for c in range(N_CORES):
        out = results[c]["y"].reshape(
            TILES, 128, NP_DIRECT + 1, 2, W).astype(np.float32)
        for t in range(TILES):
            g = TILES * c + t
            m, r0 = g // 2, (g % 2) * HALF
            blk = out[t].transpose(1, 0, 2, 3).reshape(
                NP_DIRECT + 1, HALF, W)
            K[m, :NP_DIRECT, r0:r0 + HALF] = blk[:NP_DIRECT]
            G[m, r0:r0 + HALF] = blk[NP_DIRECT]
    # Planes 12..23: plane 23-j is plane j translated by (dy-2, dx-2);
    # border pixels (where the translated source is out of bounds) are G.
    # Pure replication of device-computed values.
    for j in range(NP_DIRECT):
        dy, dx = OFFSETS[j]
        dh, dw = dy - 2, dx - 2
        a, b = max(0, dh), H + min(0, dh)
        c0, d0 = max(0, dw), W + min(0, dw)
        dst = K[:, 23 - j]
        dst[:, a:b, c0:d0] = K[:, j, a - dh:b - dh, c0 - dw:d0 - dw]
        if a > 0:
            dst[:, :a, :] = G[:, :a, :]
        if b < H:
            dst[:, b:, :] = G[:, b:, :]
        if c0 > 0:
            dst[:, a:b, :c0] = G[:, a:b, :c0]
        if d0 < W:
            dst[:, a:b, d0:] = G[:, a:b, d0:]
    return K.reshape(B, C, 24, H, W)


def run(X: np.ndarray, trace: bool = False):
    nc = _get_bass()
    in_maps = _shard_inputs(X)
    res = run_bass_kernel_spmd(nc, in_maps, list(range(N_CORES)), trace=trace)
    return _unshard_outputs(res.results), res


def kernel(X: np.ndarray) -> np.ndarray:
    out, _ = run(X, trace=False)
    return out


# revision 11
# speedup vs baseline: 2.2883x; 2.2883x over previous
"""GaussianMask kernel for Trainium2 (Bass/Tile), SPMD over 8 NeuronCores.

Problem: X [4,3,512,512] f32 -> K [4,3,24,512,512] f32 where
  K[b,c,k,h,w] = exp(-0.5 * (Xpad[b,c,h+dy,w+dx] - X[b,c,h,w])^2)
for the 24 5x5 neighbor offsets (center excluded), zero padding of 2.

Key algebra exploited on-device:

1. Offset symmetry. Offsets pair up as (dy,dx) <-> (4-dy,4-dx); plane
   23-j is plane j translated by (dy-2, dx-2), and every out-of-bounds
   border pixel of ANY plane equals G := exp(-0.5*X^2). So the device
   computes only planes 0..11 (whose dy is 0..2) plus one G plane; the
   host replicates values into planes 12..23 during unshard (pure data
   movement, no host arithmetic).

2. Gaussian via a single activation. erf'(x) = (2/sqrt(pi))*exp(-x^2),
   so exp(-0.5 d^2) = sqrt(pi)/2 * Derivative_Erf(d/sqrt(2)). The ACT
   free input scale handles 1/sqrt(2); a 4x-mode DVE tensor_scalar_mul
   applies sqrt(pi)/2. This removes the DVE squaring pass entirely:
   per plane the DVE does ONE tensor_sub (2x packed fp16 mode).

Layout (per core): 12 images x 512 rows -> 24 half-images of 256 rows;
3 per core. Partition p holds padded rows 2p..2p+3 (its 2 output rows
plus the dy=0..2 halo) of the 516-wide padded image, fp16. A second
slab loaded at +1 element keeps odd-dx reads 4B-aligned for the DVE
packed mode. Everything is fp16 (ample for the 2e-2 gate; measured l2
rel err ~2e-4), halving both DVE time and store traffic vs f32.

DMA budget: 2 loads + 6 stores = 8 HWDGE DMAs -> no DMA lane reuse.
Each instruction needs at most one cross-engine sem wait (walrus can
encode only one on DVE/DMA instructions).
"""

import numpy as np

import concourse.bass as bass
import concourse.mybir as mybir
import concourse.tile as tile
from concourse.bass_utils import run_bass_kernel_spmd

N_CORES = 8
B, C, H, W = 4, 3, 512, 512
PAD = 2
PW = W + 2 * PAD          # 516 padded width
HALF = 256                # rows per half-image tile
TILES = 3                 # half-images per core
SLAB_ROWS = 4             # padded rows 2p..2p+3 per partition
SLAB = SLAB_ROWS * PW     # 2064 elems per partition per (tile, shift)
IN_ROWS = HALF + 2        # 258 padded rows per half-image
IN_TILE = IN_ROWS * PW    # 133128 elems per half-image input
IN_LEN = TILES * IN_TILE + 8   # +8 pad so the +1-shifted load stays in bounds
NP_DIRECT = 12            # planes computed on device
CHUNK = (NP_DIRECT + 1) * 2 * W   # 13312 out elems per partition per tile
HALF_A = 6 * 2 * W        # 6144: planes 0..5
HALF_B = CHUNK - HALF_A   # 7168: planes 6..11 + G

INV_SQRT2 = 0.7071067811865476
SQRT_PI_OVER_2 = 0.8862269254527580

# planes 0..11 = reference planes 0..11 (idx k for k < 12)
OFFSETS = [(k // 5, k % 5) for k in range(NP_DIRECT)]

_CACHED = None


def _patch_tail_drain():
    """Split the kernel-tail drain's sem waits across one drain per sem.

    Tile attaches every outstanding semaphore wait to a single Drain
    instruction, but walrus' CTRL codegen can only encode a bounded
    number of sync waits per instruction and dies with "Too many sync
    wait commands". One drain per nonzero proc keeps every instruction
    at a single wait.
    """
    from concourse.tile import TileContext
    from concourse.vector_clock import ScopedClock, VectorClock

    if getattr(TileContext, "_tail_drain_patched", False):
        return

    def _drain_and_barrier(self, tick_clock, wait_clock):
        gc = tick_clock.global_clock
        vals = eval(repr(gc).replace("VectorClock", ""))
        for i, v in enumerate(vals):
            if v <= 0:
                continue
            sub = [0] * len(vals)
            sub[i] = v
            drain_inst = self.nc.sync.drain()
            wait_clock.add_sem_waits(
                drain_inst.ins, ScopedClock({None: VectorClock(sub)}))
        self.nc.all_engine_barrier()
        assert self.sems is not None
        popped = self.nc._tile_sem_poison_stack.pop()
        assert popped is self._sem_poison
        self.nc.clear_and_free_semaphores(list(self.sems.allocated().values()))
        self.nc.all_engine_barrier()

    TileContext._drain_and_barrier = _drain_and_barrier
    TileContext._tail_drain_patched = True


def _build_bass():
    _patch_tail_drain()
    nc = bass.Bass("TRN2", target_bir_lowering=False, debug=False,
                   num_devices=N_CORES, dynamic_dma_scratch_size=4096)
    x_h = nc.dram_tensor("x", [IN_LEN], mybir.dt.float16,
                         kind="ExternalInput")
    y_h = nc.dram_tensor("y", [TILES * 128 * CHUNK], mybir.dt.float16,
                         kind="ExternalOutput")

    f16 = mybir.dt.float16
    DErf = mybir.ActivationFunctionType.Derivative_Erf

    with tile.TileContext(nc) as tc:
        with (
            tc.tile_pool(name="slab", bufs=1) as ps,
            tc.tile_pool(name="d", bufs=6) as pd,
            tc.tile_pool(name="e", bufs=2) as pe,
            tc.tile_pool(name="out", bufs=6) as po,
            tc.tile_pool(name="scratch", bufs=1) as psc,
        ):
            # slab layout per partition: [tile 3][elem 2064]; the _o copy
            # is loaded at +1 element so odd-dx reads stay 4B-aligned.
            load_dims = [[2 * PW, 128], [IN_TILE, TILES], [1, SLAB]]
            slab_e = ps.tile([128, TILES * SLAB], f16, tag="se")
            nc.sync.dma_start(
                out=slab_e[:].rearrange("p (t e) -> p t e", e=SLAB),
                in_=bass.AP(x_h, 0, load_dims))
            slab_o = ps.tile([128, TILES * SLAB], f16, tag="so")
            nc.sync.dma_start(
                out=slab_o[:].rearrange("p (t e) -> p t e", e=SLAB),
                in_=bass.AP(x_h, 1, load_dims))

            prev_act = None

            def chain_act(inst):
                # Pin the ACT queue to program order: each chunk's absorber
                # must precede the next chunks' DErf so its observed tick
                # elides their e-buf WAW/WAR (one sem wait per instruction).
                nonlocal prev_act
                if prev_act is not None:
                    tile.add_dep_helper(inst.ins, prev_act.ins, sync=False,
                                        reason="act program order")
                prev_act = inst
                return inst

            for t in range(TILES):
                ve = slab_e[:, t * SLAB:(t + 1) * SLAB].rearrange(
                    "p (r c) -> p r c", c=PW)
                vo = slab_o[:, t * SLAB:(t + 1) * SLAB].rearrange(
                    "p (r c) -> p r c", c=PW)
                xi = ve[:, 2:4, 2:2 + W]

                for half in range(2):
                    h = 2 * t + half
                    ncols = HALF_A if half == 0 else HALF_B
                    d = pd.tile([128, HALF_A], f16, tag="d")
                    for j in range(6):
                        dy, dx = OFFSETS[half * 6 + j]
                        if dx % 2 == 0:
                            xj = ve[:, dy:dy + 2, dx:dx + W]
                        else:
                            xj = vo[:, dy:dy + 2, dx - 1:dx - 1 + W]
                        nc.vector.tensor_sub(
                            d[:, j * 1024:(j + 1) * 1024].rearrange(
                                "p (r c) -> p r c", c=W), xj, xi)
                    # d -> (ACT DErf) -> e -> (DVE x sqrt(pi)/2) -> o -> DMA.
                    # e and o keep a single writer engine each, so the TS and
                    # the store need exactly one sem wait.
                    e = pe.tile([128, HALF_B], f16, tag="e")
                    chain_act(nc.scalar.activation(e[:, 0:HALF_A], d[:],
                                                   DErf, scale=INV_SQRT2))
                    if half == 1:
                        chain_act(nc.scalar.activation(
                            e[:, HALF_A:HALF_B].rearrange(
                                "p (r c) -> p r c", c=W),
                            xi, DErf, scale=INV_SQRT2))
                    # absorber: a tiny ACT op reading the tail of every ACT
                    # write of this chunk gets a single self-sem wait, which
                    # advances the ACT engine's observed clock so the e-buf
                    # WAW of chunk h+2's DErf is elided (each instruction can
                    # encode only ONE sem wait).
                    sa = psc.tile([128, 4], f16, tag="sa")
                    span = 4 if half == 1 else 2
                    chain_act(nc.scalar.copy(
                        sa[:, 0:span], e[:, HALF_A - 2:HALF_A - 2 + span]))
                    o = po.tile([128, HALF_B], f16, tag="o")
                    nc.vector.tensor_scalar_mul(o[:, 0:ncols], e[:, 0:ncols],
                                                SQRT_PI_OVER_2)
                    dst = bass.AP(y_h, (t * 128) * CHUNK + half * HALF_A,
                                  [[CHUNK, 128], [1, ncols]])
                    nc.sync.dma_start(out=dst, in_=o[:, 0:ncols])
    return nc


def _get_bass():
    global _CACHED
    if _CACHED is None:
        _CACHED = _build_bass()
    return _CACHED


def _shard_inputs(X: np.ndarray):
    """Full X [4,3,512,512] -> per-core flat padded half-image stacks (fp16)."""
    Xi = np.ascontiguousarray(X, dtype=np.float32).reshape(B * C, H, W)
    Xp = np.pad(Xi, ((0, 0), (PAD, PAD), (PAD, PAD))).astype(np.float16)
    in_maps = []
    for c in range(N_CORES):
        arr = np.zeros([IN_LEN], dtype=np.float16)
        for t in range(TILES):
            g = TILES * c + t
            m, r0 = g // 2, (g % 2) * HALF
            arr[t * IN_TILE:(t + 1) * IN_TILE] = \
                Xp[m, r0:r0 + IN_ROWS, :].reshape(-1)
        in_maps.append({"x": arr})
    return in_maps


def _unshard_outputs(results):
    K = np.empty((B * C, 24, H, W), dtype=np.float32)
    G = np.empty((B * C, H, W), dtype=np.float32)
    for c in range(N_CORES):
        out = results[c]["y"].reshape(
            TILES, 128, NP_DIRECT + 1, 2, W).astype(np.float32)
        for t in range(TILES):
            g = TILES * c + t
            m, r0 = g // 2, (g % 2) * HALF
            blk = out[t].transpose(1, 0, 2, 3).reshape(
                NP_DIRECT + 1, HALF, W)
            K[m, :NP_DIRECT, r0:r0 + HALF] = blk[:NP_DIRECT]
            G[m, r0:r0 + HALF] = blk[NP_DIRECT]
    # Planes 12..23: plane 23-j is plane j translated by (dy-2, dx-2);
    # border pixels (where the translated source is out of bounds) are G.
    # Pure replication of device-computed values.
    for j in range(NP_DIRECT):
        dy, dx = OFFSETS[j]
        dh, dw = dy - 2, dx - 2
        a, b = max(0, dh), H + min(0, dh)
        c0, d0 = max(0, dw), W + min(0, dw)
        dst = K[:, 23 - j]
        dst[:, a:b, c0:d0] = K[:, j, a - dh:b - dh, c0 - dw:d0 - dw]
        if a > 0:
            dst[:, :a, :] = G[:, :a, :]
        if b < H:
            dst[:, b:, :] = G[:, b:, :]
        if c0 > 0:
            dst[:, a:b, :c0] = G[:, a:b, :c0]
        if d0 < W:
            dst[:, a:b, d0:] = G[:, a:b, d0:]
    return K.reshape(B, C, 24, H, W)


def run(X: np.ndarray, trace: bool = False):
    nc = _get_bass()
    in_maps = _shard_inputs(X)
    res = run_bass_kernel_spmd(nc, in_maps, list(range(N_CORES)), trace=trace)
    return _unshard_outputs(res.results), res


def kernel(X: np.ndarray) -> np.ndarray:
    out, _ = run(X, trace=False)
    return out


# revision 16
# speedup vs baseline: 2.4752x; 1.0817x over previous
"""GaussianMask kernel for Trainium2 (Bass/Tile), SPMD over 8 NeuronCores.

Problem: X [4,3,512,512] f32 -> K [4,3,24,512,512] f32 where
  K[b,c,k,h,w] = exp(-0.5 * (Xpad[b,c,h+dy,w+dx] - X[b,c,h,w])^2)
for the 24 5x5 neighbor offsets (center excluded), zero padding of 2.

Key algebra exploited on-device:

1. Offset symmetry. Offsets pair up as (dy,dx) <-> (4-dy,4-dx); plane
   23-j is plane j translated by (dy-2, dx-2), and every out-of-bounds
   border pixel of ANY plane equals G := exp(-0.5*X^2). So the device
   computes only planes 0..11 (whose dy is 0..2) plus one G plane; the
   host replicates values into planes 12..23 during unshard (pure data
   movement, no host arithmetic).

2. Gaussian via a single activation. erf'(x) = (2/sqrt(pi))*exp(-x^2),
   so exp(-0.5 d^2) = sqrt(pi)/2 * Derivative_Erf(d/sqrt(2)). The ACT
   free input scale handles 1/sqrt(2); a 4x-mode DVE tensor_scalar_mul
   applies sqrt(pi)/2. This removes the DVE squaring pass entirely:
   per plane the DVE does ONE tensor_sub (2x packed fp16 mode).

Layout (per core): 12 images x 512 rows -> 24 half-images of 256 rows;
3 per core. Partition p holds padded rows 2p..2p+3 (its 2 output rows
plus the dy=0..2 halo) of the 516-wide padded image, fp16. A second
slab loaded at +1 element keeps odd-dx reads 4B-aligned for the DVE
packed mode. Everything is fp16 (ample for the 2e-2 gate; measured l2
rel err ~2e-4), halving both DVE time and store traffic vs f32.

DMA budget: 2 loads + 6 stores = 8 HWDGE DMAs -> no DMA lane reuse.
Each instruction needs at most one cross-engine sem wait (walrus can
encode only one on DVE/DMA instructions).
"""

import numpy as np

import concourse.bass as bass
import concourse.mybir as mybir
import concourse.tile as tile
from concourse.bass_utils import run_bass_kernel_spmd

N_CORES = 8
B, C, H, W = 4, 3, 512, 512
PAD = 2
PW = W + 2 * PAD          # 516 padded width
HALF = 256                # rows per half-image tile
TILES = 3                 # half-images per core
SLAB_ROWS = 4             # padded rows 2p..2p+3 per partition
SLAB = SLAB_ROWS * PW     # 2064 elems per partition per (tile, shift)
IN_ROWS = HALF + 2        # 258 padded rows per half-image
IN_TILE = IN_ROWS * PW    # 133128 elems per half-image input
# x layout: [tile0][t1_e][t1_o][t2_e][t2_o] — tiles 1-2 are duplicated by
# the host (o = e shifted one element) so a single 3-dim DMA can load all
# four blocks; tile 0 instead uses an overlapping-read AP and its own
# (first) DMA so compute starts after ~1MB.
IN_LEN = 5 * IN_TILE + 8
NP_DIRECT = 12            # planes computed on device
CHUNK = (NP_DIRECT + 1) * 2 * W   # 13312 out elems per partition per tile
HALF_A = 6 * 2 * W        # 6144: planes 0..5
HALF_B = CHUNK - HALF_A   # 7168: planes 6..11 + G

INV_SQRT2 = 0.7071067811865476
SQRT_PI_OVER_2 = 0.8862269254527580

# planes 0..11 = reference planes 0..11 (idx k for k < 12)
OFFSETS = [(k // 5, k % 5) for k in range(NP_DIRECT)]

_CACHED = None


def _patch_tail_drain():
    """Split the kernel-tail drain's sem waits across one drain per sem.

    Tile attaches every outstanding semaphore wait to a single Drain
    instruction, but walrus' CTRL codegen can only encode a bounded
    number of sync waits per instruction and dies with "Too many sync
    wait commands". One drain per nonzero proc keeps every instruction
    at a single wait.
    """
    from concourse.tile import TileContext
    from concourse.vector_clock import ScopedClock, VectorClock

    if getattr(TileContext, "_tail_drain_patched", False):
        return

    def _drain_and_barrier(self, tick_clock, wait_clock):
        gc = tick_clock.global_clock
        vals = eval(repr(gc).replace("VectorClock", ""))
        for i, v in enumerate(vals):
            if v <= 0:
                continue
            sub = [0] * len(vals)
            sub[i] = v
            drain_inst = self.nc.sync.drain()
            wait_clock.add_sem_waits(
                drain_inst.ins, ScopedClock({None: VectorClock(sub)}))
        self.nc.all_engine_barrier()
        assert self.sems is not None
        popped = self.nc._tile_sem_poison_stack.pop()
        assert popped is self._sem_poison
        self.nc.clear_and_free_semaphores(list(self.sems.allocated().values()))
        self.nc.all_engine_barrier()

    TileContext._drain_and_barrier = _drain_and_barrier
    TileContext._tail_drain_patched = True


def _build_bass():
    _patch_tail_drain()
    nc = bass.Bass("TRN2", target_bir_lowering=False, debug=False,
                   num_devices=N_CORES, dynamic_dma_scratch_size=4096)
    x_h = nc.dram_tensor("x", [IN_LEN], mybir.dt.float16,
                         kind="ExternalInput")
    y_h = nc.dram_tensor("y", [TILES * 128 * CHUNK], mybir.dt.float16,
                         kind="ExternalOutput")

    f16 = mybir.dt.float16
    DErf = mybir.ActivationFunctionType.Derivative_Erf

    # Per-tile chunk plan: lists of (first_plane, n_planes, has_g). First
    # chunk small (ACT spine starts early), last chunk small (short tail),
    # middle chunks big (fewer stores; 2 loads + 6 stores = 8 DMAs, the
    # hard cap before lane reuse forces a second sem wait on a DMA).
    PLANS = [
        [(0, 4, False), (4, 8, True)],
        [(0, 9, False), (9, 3, True)],
        [(0, 9, False), (9, 3, True)],
    ]

    with tile.TileContext(nc) as tc:
        with (
            tc.tile_pool(name="slab", bufs=1) as ps,
            tc.tile_pool(name="d4", bufs=1) as pd4,
            tc.tile_pool(name="d8", bufs=1) as pd8,
            tc.tile_pool(name="d9", bufs=2) as pd9,
            tc.tile_pool(name="d3", bufs=2) as pd3,
            tc.tile_pool(name="e", bufs=2) as pe,
            tc.tile_pool(name="o4", bufs=3) as po4,
            tc.tile_pool(name="o9", bufs=3) as po9,
            tc.tile_pool(name="scratch", bufs=1) as psc,
        ):
            dpools = {4: pd4, 8: pd8, 9: pd9, 3: pd3}
            opools = {4: po4, 9: po9}
            EMAX = 9 * 2 * W  # 9216: biggest chunk (units incl G)

            # One slab tile per partition: [tile 3][shift 2][elem 2064]; the
            # shift-1 copy reads at +1 element so odd-dx views stay
            # 4B-aligned for the DVE packed mode. Tile 0 is its own (first)
            # DMA so compute can start after ~1MB instead of ~3MB.
            slab = ps.tile([128, TILES * 2 * SLAB], f16, tag="slab")
            nc.sync.dma_start(
                out=slab[:, 0:2 * SLAB].rearrange("p (s e) -> p s e", e=SLAB),
                in_=bass.AP(x_h, 0, [[2 * PW, 128], [1, 2], [1, SLAB]]))
            nc.sync.dma_start(
                out=slab[:, 2 * SLAB:].rearrange("p (b e) -> p b e", e=SLAB),
                in_=bass.AP(x_h, IN_TILE,
                            [[2 * PW, 128], [IN_TILE, 4], [1, SLAB]]))

            prev_act = None
            prev_sub = None

            def chain_act(inst):
                # Pin the ACT queue to program order: each chunk's absorber
                # must precede the next chunks' DErf so its observed tick
                # elides their e-buf WAW/WAR (one sem wait per instruction).
                nonlocal prev_act
                if prev_act is not None:
                    tile.add_dep_helper(inst.ins, prev_act.ins, sync=False,
                                        reason="act program order")
                prev_act = inst
                return inst

            def chain_sub(inst):
                # Pin the subs to program order so each DErf's DVE wait is
                # exactly its own chunk's last sub (the greedy scheduler
                # otherwise interleaves chunks and inflates the wait).
                nonlocal prev_sub
                if prev_sub is not None:
                    tile.add_dep_helper(inst.ins, prev_sub.ins, sync=False,
                                        reason="sub program order")
                prev_sub = inst
                return inst

            for t in range(TILES):
                ve = slab[:, (2 * t) * SLAB:(2 * t + 1) * SLAB].rearrange(
                    "p (r c) -> p r c", c=PW)
                vo = slab[:, (2 * t + 1) * SLAB:(2 * t + 2) * SLAB].rearrange(
                    "p (r c) -> p r c", c=PW)
                xi = ve[:, 2:4, 2:2 + W]

                for (p0, np_, has_g) in PLANS[t]:
                    units = np_ + (1 if has_g else 0)
                    ncols = units * 2 * W
                    dcols = np_ * 2 * W
                    d = dpools[np_].tile([128, dcols], f16, tag=f"d{np_}")
                    for j in range(np_):
                        dy, dx = OFFSETS[p0 + j]
                        if dx % 2 == 0:
                            xj = ve[:, dy:dy + 2, dx:dx + W]
                        else:
                            xj = vo[:, dy:dy + 2, dx - 1:dx - 1 + W]
                        chain_sub(nc.vector.tensor_sub(
                            d[:, j * 1024:(j + 1) * 1024].rearrange(
                                "p (r c) -> p r c", c=W), xj, xi))
                    # d -> (ACT DErf) -> e -> (DVE x sqrt(pi)/2) -> o -> DMA.
                    # e and o keep a single writer engine each, so the TS and
                    # the store need exactly one sem wait.
                    e = pe.tile([128, EMAX], f16, tag="e")
                    chain_act(nc.scalar.activation(e[:, 0:dcols], d[:],
                                                   DErf, scale=INV_SQRT2))
                    if has_g:
                        chain_act(nc.scalar.activation(
                            e[:, dcols:ncols].rearrange(
                                "p (r c) -> p r c", c=W),
                            xi, DErf, scale=INV_SQRT2))
                    # absorber: a tiny ACT op reading the tail of every ACT
                    # write of this chunk gets a single self-sem wait, which
                    # advances the ACT engine's observed clock so the e-buf
                    # WAW of later chunks' DErf is elided (each instruction
                    # can encode only ONE sem wait).
                    sa = psc.tile([128, 4], f16, tag="sa")
                    span = 4 if has_g else 2
                    chain_act(nc.scalar.copy(
                        sa[:, 0:span], e[:, dcols - 2:dcols - 2 + span]))
                    o = opools[units].tile([128, ncols], f16, tag=f"o{units}")
                    nc.vector.tensor_scalar_mul(o[:], e[:, 0:ncols],
                                                SQRT_PI_OVER_2)
                    dst = bass.AP(y_h, (t * 128) * CHUNK + p0 * 2 * W,
                                  [[CHUNK, 128], [1, ncols]])
                    nc.sync.dma_start(out=dst, in_=o[:])
    return nc


def _get_bass():
    global _CACHED
    if _CACHED is None:
        _CACHED = _build_bass()
    return _CACHED


def _shard_inputs(X: np.ndarray):
    """Full X [4,3,512,512] -> per-core flat padded half-image stacks (fp16).

    Layout: [tile0][t1_e][t1_o][t2_e][t2_o]; the _o blocks are the _e
    blocks shifted one element so the kernel's single 3-dim DMA gets
    4B-aligned odd-dx views.
    """
    Xi = np.ascontiguousarray(X, dtype=np.float32).reshape(B * C, H, W)
    Xp = np.pad(Xi, ((0, 0), (PAD, PAD), (PAD, PAD))).astype(np.float16)
    in_maps = []
    for c in range(N_CORES):
        arr = np.zeros([IN_LEN], dtype=np.float16)

        def block(t):
            g = TILES * c + t
            m, r0 = g // 2, (g % 2) * HALF
            return Xp[m, r0:r0 + IN_ROWS, :].reshape(-1)

        arr[0:IN_TILE] = block(0)
        for j, (t, s) in enumerate([(1, 0), (1, 1), (2, 0), (2, 1)]):
            blk = block(t)
            off = (1 + j) * IN_TILE
            if s == 0:
                arr[off:off + IN_TILE] = blk
            else:
                arr[off:off + IN_TILE - 1] = blk[1:]
        in_maps.append({"x": arr})
    return in_maps


def _unshard_outputs(results):
    K = np.empty((B * C, 24, H, W), dtype=np.float32)
    G = np.empty((B * C, H, W), dtype=np.float32)
    for c in range(N_CORES):
        out = results[c]["y"].reshape(
            TILES, 128, NP_DIRECT + 1, 2, W).astype(np.float32)
        for t in range(TILES):
            g = TILES * c + t
            m, r0 = g // 2, (g % 2) * HALF
            blk = out[t].transpose(1, 0, 2, 3).reshape(
                NP_DIRECT + 1, HALF, W)
            K[m, :NP_DIRECT, r0:r0 + HALF] = blk[:NP_DIRECT]
            G[m, r0:r0 + HALF] = blk[NP_DIRECT]
    # Planes 12..23: plane 23-j is plane j translated by (dy-2, dx-2);
    # border pixels (where the translated source is out of bounds) are G.
    # Pure replication of device-computed values.
    for j in range(NP_DIRECT):
        dy, dx = OFFSETS[j]
        dh, dw = dy - 2, dx - 2
        a, b = max(0, dh), H + min(0, dh)
        c0, d0 = max(0, dw), W + min(0, dw)
        dst = K[:, 23 - j]
        dst[:, a:b, c0:d0] = K[:, j, a - dh:b - dh, c0 - dw:d0 - dw]
        if a > 0:
            dst[:, :a, :] = G[:, :a, :]
        if b < H:
            dst[:, b:, :] = G[:, b:, :]
        if c0 > 0:
            dst[:, a:b, :c0] = G[:, a:b, :c0]
        if d0 < W:
            dst[:, a:b, d0:] = G[:, a:b, d0:]
    return K.reshape(B, C, 24, H, W)


def run(X: np.ndarray, trace: bool = False):
    nc = _get_bass()
    in_maps = _shard_inputs(X)
    res = run_bass_kernel_spmd(nc, in_maps, list(range(N_CORES)), trace=trace)
    return _unshard_outputs(res.results), res


def kernel(X: np.ndarray) -> np.ndarray:
    out, _ = run(X, trace=False)
    return out
